# revision 1
# baseline (speedup 1.0000x reference)
"""DAGNN (MLP + 10-hop SpMM propagation + sigmoid-gated combine) on 8 trn2
NeuronCores via Bass/Tile.

Strategy:
  - dst-sharding: core c owns 12500 destination nodes.
  - MLP h0 = relu(relu(x@W1)@W2) computed per-core on its dst slice (PE, f32),
    x passed pre-transposed (host) so no on-device transposes are needed.
  - Node-feature table [100000, 128] bf16 (64 real features + 64 pad to reach
    the 256B dma_gather element size) replicated per-core in DRAM, rebuilt
    each hop with an AllGather of the 8 per-core slices.
  - Per hop: per-edge rows fetched with dma_gather (InstDMAGatherAnt, 4 SWDGE
    queues, one per 25000-row src bank so indices fit int16), then a one-hot
    weighted segment-sum on the TensorEngine: out[win] += S.T @ G with S
    [128 edges, 32 dsts] bf16 streamed from DRAM, accumulating f32 in PSUM
    windows at partition offsets {0,32,64,96}.
  - Canonical per-group structure (4 superblocks of 128 dsts): per bank one
    M-tile per 32-window plus two R-tiles for bucket overflow, so the
    instruction stream is identical across cores (SPMD) and only idx/S data
    differ.
  - Gating acc += sigmoid(h_k . w_prop) * h_k folded into each hop's PSUM
    eviction (DVE/ACT), acc kept f32 in SBUF.
  - Host gathers out[node_index] from the 8 returned slices.
"""

import math

import numpy as np
import ml_dtypes

import concourse.bass as bass
import concourse.bacc as bacc
import concourse.tile as tile
import concourse.mybir as mybir
from concourse import library_config
from concourse.tile import add_dep_helper

F32 = mybir.dt.float32
BF16 = mybir.dt.bfloat16
I16 = mybir.dt.int16

# problem constants (hardcoded per harness contract)
N_NODES = 100000
N_EDGES = 1600000
K_HOPS = 10
D_IN = 512
D_HID = 64
N_IDX = 10000
N_CORES = 8


# ---------------------------------------------------------------------------
# structure (shared across cores; sizes must be core-independent)
# ---------------------------------------------------------------------------
class Struct:
    def __init__(self, n_nodes, n_cores, n_banks=4, sb_per_group=4, r_tiles=2, hops=K_HOPS):
        self.n_nodes = n_nodes
        self.hops = hops
        self.n_cores = n_cores
        self.shard = n_nodes // n_cores
        assert self.shard * n_cores == n_nodes
        self.n_banks = n_banks
        self.nsb = int(math.ceil(self.shard / 128))
        self.bank_sz = int(math.ceil(n_nodes / n_banks))
        assert self.bank_sz <= 32768
        self.r_tiles = r_tiles
        # groups of superblocks
        self.groups = [
            list(range(g, min(g + sb_per_group, self.nsb)))
            for g in range(0, self.nsb, sb_per_group)
        ]
        # windows per sb: list of (sb, woff, ndst_in_window)
        self.sb_ndst = [
            min(128, self.shard - sb * 128) for sb in range(self.nsb)
        ]
        self.sb_wins = []
        for sb in range(self.nsb):
            wins = []
            nd = self.sb_ndst[sb]
            for woff in range(0, nd, 32):
                wins.append((woff, min(32, nd - woff)))
            self.sb_wins.append(wins)
        # per group: windows list [(sb_local, sb, woff)], tiles per bank
        self.g_wins = []
        self.g_tiles = []  # tiles per bank per group (nwin + r_tiles)
        for sbs in self.groups:
            wins = []
            for sl, sb in enumerate(sbs):
                for (woff, _n) in self.sb_wins[sb]:
                    wins.append((sl, sb, woff))
            self.g_wins.append(wins)
            self.g_tiles.append(len(wins) + r_tiles)
        self.max_tiles = max(self.g_tiles)
        # matmuls per group: banks * nwin (M) + banks * r_tiles * nwin (R)
        self.g_mms = [
            self.n_banks * len(w) * (1 + r_tiles) for w in self.g_wins
        ]
        self.max_mms = max(self.g_mms)
        self.s_cols = self.max_mms * 32  # S columns per group (padded)
        self.n_groups = len(self.groups)
        # idx tensor: per (group, bank) call: max_tiles*128 idxs -> /16 cols
        self.idx_cols = self.max_tiles * 8  # (tiles*128)/16


# ---------------------------------------------------------------------------
# host-side data prep
# ---------------------------------------------------------------------------
def prep_core(st: Struct, c, edge_src, edge_dst, edge_weight):
    """Build idx layout + S tiles for one core. Returns (idx_all, s_all)."""
    lo, hi = c * st.shard, (c + 1) * st.shard
    m = (edge_dst >= lo) & (edge_dst < hi)
    src = edge_src[m].astype(np.int64)
    dstl = (edge_dst[m] - lo).astype(np.int64)
    w = edge_weight[m].astype(np.float32)
    bank = src // st.bank_sz
    srcl = src - bank * st.bank_sz

    sb = dstl >> 7
    woff = (dstl & 127) & ~31  # 32-aligned offset within sb

    idx_all = np.zeros((st.n_groups, st.n_banks, 128, st.idx_cols), np.int16)
    s_all = np.zeros((st.n_groups, 128, st.s_cols), np.float32)

    # bucket edges by (group, bank, window-within-group)
    g_of_sb = np.zeros(st.nsb, np.int64)
    for gi, sbs in enumerate(st.groups):
        for s in sbs:
            g_of_sb[s] = gi
    g = g_of_sb[sb]

    order = np.lexsort((dstl, bank, g))
    src_o, dstl_o, w_o, bank_o, g_o = (
        srcl[order], dstl[order], w[order], bank[order], g[order])
    sb_o, woff_o = sb[order], woff[order]

    # boundaries per (g, bank)
    key = g_o * st.n_banks + bank_o
    bounds = np.searchsorted(key, np.arange(st.n_groups * st.n_banks + 1))

    for gi in range(st.n_groups):
        wins = st.g_wins[gi]
        win_pos = {(sbv, wo): k for k, (_sl, sbv, wo) in enumerate(wins)}
        nwin = len(wins)
        n_mm_g = st.n_banks * nwin * (1 + st.r_tiles)
        mm = 0
        # slot layout per bank: [win0 tile | win1 tile | ... | R0 | R1]
        for b in range(st.n_banks):
            a0, a1 = bounds[gi * st.n_banks + b], bounds[gi * st.n_banks + b + 1]
            es, ed, ew2 = src_o[a0:a1], dstl_o[a0:a1], w_o[a0:a1]
            esb, ewo = sb_o[a0:a1], woff_o[a0:a1]
            # per window lists
            rest_idx = []
            rest_dst = []
            rest_w = []
            tiles_idx = np.zeros((st.max_tiles, 128), np.int16)
            tiles_S = [None] * st.max_tiles  # (rows, cols, vals) per tile
            for k, (_sl, sbv, wo) in enumerate(wins):
                emask = (esb == sbv) & (ewo == wo)
                ii = np.nonzero(emask)[0]
                take = ii[:128]
                over = ii[128:]
                n = len(take)
                tiles_idx[k, :n] = es[take]
                tiles_S[k] = (np.arange(n), ed[take] - (sbv * 128 + wo),
                              ew2[take])
                if len(over):
                    rest_idx.append(es[over])
                    rest_dst.append(ed[over])
                    rest_w.append(ew2[over])
            if rest_idx:
                rest_idx = np.concatenate(rest_idx)
                rest_dst = np.concatenate(rest_dst)
                rest_w = np.concatenate(rest_w)
            else:
                rest_idx = np.zeros(0, np.int64)
                rest_dst = np.zeros(0, np.int64)
                rest_w = np.zeros(0, np.float32)
            assert len(rest_idx) <= 128 * st.r_tiles, (
                f"R overflow core{c} g{gi} b{b}: {len(rest_idx)}")
            for rt in range(st.r_tiles):
                rr = slice(rt * 128, min((rt + 1) * 128, len(rest_idx)))
                ridx = rest_idx[rr]
                k = nwin + rt
                tiles_idx[k, :len(ridx)] = ridx
                tiles_S[k] = (rest_dst[rr], rest_w[rr])

            # idx layout: tile t slot p -> linear j = t*128+p ->
            # [partition j%16 (replicated x8 later), col j//16]
            lin = tiles_idx.reshape(-1)  # [max_tiles*128]
            wrapped = lin.reshape(-1, 16).T  # [16, idx_cols]
            idx_all[gi, b] = np.tile(wrapped, (8, 1))

            # S blocks: M then (R x r_tiles), emission order bank-major
            # M blocks for this bank at mm positions: b*nwin + k
            for k in range(nwin):
                rows, cols, vals = tiles_S[k]
                pos = b * nwin + k
                blk = np.zeros((128, 32), np.float32)
                blk[rows, cols] = vals
                s_all[gi, :, pos * 32:(pos + 1) * 32] = blk
            # R blocks: position banks*nwin + (rt*banks + b)*nwin + k
            for rt in range(st.r_tiles):
                rdst, rw = tiles_S[nwin + rt]
                for k, (_sl, sbv, wo) in enumerate(wins):
                    pos = (st.n_banks * nwin
                           + (rt * st.n_banks + b) * nwin + k)
                    blk = np.zeros((128, 32), np.float32)
                    base = sbv * 128 + wo
                    sel = (rdst >= base) & (rdst < base + 32)
                    rr = np.nonzero(sel)[0]
                    blk[rr, rdst[rr] - base] = rw[rr]
                    s_all[gi, :, pos * 32:(pos + 1) * 32] = blk
        del n_mm_g, mm
    # idx layout for the single [128, G*B*C] SBUF load
    idx_flat = idx_all.transpose(2, 0, 1, 3).reshape(128, -1)
    return np.ascontiguousarray(idx_flat), s_all.astype(ml_dtypes.bfloat16)


# ---------------------------------------------------------------------------
# device program
# ---------------------------------------------------------------------------
def build_nc(st: Struct, hops: int = K_HOPS, ag_mode: str = 'all'):
    nc = bacc.Bacc(
        "TRN2", target_bir_lowering=False, debug=False, enable_asserts=False,
        num_devices=st.n_cores, num_swdge_queues=st.n_banks)

    shard = st.shard
    xT = nc.dram_tensor("xT", [D_IN, shard], F32, kind="ExternalInput")
    W1 = nc.dram_tensor("W1", [D_IN, D_HID], F32, kind="ExternalInput")
    W2 = nc.dram_tensor("W2", [D_HID, D_HID], F32, kind="ExternalInput")
    wprop = nc.dram_tensor("wprop", [128, D_HID], F32, kind="ExternalInput")
    idx_d = nc.dram_tensor(
        "idx", [128, st.n_groups * st.n_banks * st.idx_cols], I16,
        kind="ExternalInput")
    s_d = nc.dram_tensor(
        "S", [st.n_groups, 128, st.s_cols], BF16, kind="ExternalInput")
    out_d = nc.dram_tensor("out", [shard, D_HID], F32, kind="ExternalOutput")

    bounces = [nc.dram_tensor(f"bounce{p}", [shard, 128], BF16)
               for p in range(2)]
    tables = [nc.dram_tensor(f"table{p}", [st.n_nodes, 128], BF16)
              for p in range(2)]

    replica = [list(range(st.n_cores))]

    with tile.TileContext(nc) as tc:
        with (
            tc.tile_pool(name="sbuf", bufs=2) as sp,
            tc.tile_pool(name="persist", bufs=1) as pp,
            tc.tile_pool(name="psum", bufs=2, space="PSUM") as qp,
        ):
            ll = nc.gpsimd.load_library(library_config.mlp)

            # persistent tiles
            acc = pp.tile([128, st.nsb * 64], F32, tag="acc")
            idx_sb = pp.tile([128, st.n_groups * st.n_banks * st.idx_cols],
                             I16, tag="idxs")
            wp_t = pp.tile([128, 64], F32, tag="wp")
            w1_t = pp.tile([128, 4 * 64], F32, tag="w1")
            w2_t = pp.tile([64, 64], F32, tag="w2")
            nc.sync.dma_start(out=wp_t[:], in_=wprop[:])
            nc.sync.dma_start(
                out=w1_t[:].rearrange("p (c d) -> p c d", d=64),
                in_=W1[:].rearrange("(c p) d -> p c d", p=128))
            nc.sync.dma_start(out=w2_t[:], in_=W2[:])
            nc.sync.dma_start(out=idx_sb[:], in_=idx_d[:])
            nc.vector.memset(acc[:], 0.0)

            def gate_and_bounce(h_f32, h_b16, gi, hop):
                """gating acc += sig(h.wp)*h ; write bounce slices (bf16)."""
                sbs = st.groups[gi]
                nsb = len(sbs)
                dot = sp.tile([128, 8], F32, tag="dot")
                sg = sp.tile([128, 8], F32, tag="sg")
                tmp = sp.tile([128, 8 * 64], F32, tag="gtmp")
                h3 = h_f32[:, 0:nsb * 64].rearrange("p (s f) -> p s f", f=64)
                nc.vector.tensor_tensor(
                    out=tmp[:, 0:nsb * 64].rearrange("p (s f) -> p s f", f=64),
                    in0=h3,
                    in1=wp_t[:].rearrange("p (o f) -> p o f",
                                          o=1).to_broadcast([128, nsb, 64]),
                    op=mybir.AluOpType.mult)
                nc.vector.tensor_reduce(
                    out=dot[:, :nsb],
                    in_=tmp[:, 0:nsb * 64].rearrange("p (s f) -> p s f",
                                                     f=64),
                    axis=mybir.AxisListType.X,
                    op=mybir.AluOpType.add)
                nc.scalar.activation(
                    sg[:, :nsb], dot[:, :nsb],
                    mybir.ActivationFunctionType.Sigmoid)
                nc.vector.tensor_tensor(
                    out=tmp[:, 0:nsb * 64].rearrange("p (s f) -> p s f", f=64),
                    in0=h3,
                    in1=sg[:, :nsb].rearrange("p (s o) -> p s o",
                                              o=1).to_broadcast(
                        [128, nsb, 64]),
                    op=mybir.AluOpType.mult)
                g0 = sbs[0]
                nc.vector.tensor_tensor(
                    out=acc[:, g0 * 64:(g0 + nsb) * 64],
                    in0=acc[:, g0 * 64:(g0 + nsb) * 64],
                    in1=tmp[:, 0:nsb * 64],
                    op=mybir.AluOpType.add)
                if hop < st.hops:
                    bnc = bounces[hop % 2]
                    for sl, sbv in enumerate(sbs):
                        nd = st.sb_ndst[sbv]
                        nc.sync.dma_start(
                            out=bnc[sbv * 128:sbv * 128 + nd, 0:64],
                            in_=h_b16[0:nd, sl * 64:(sl + 1) * 64])

            # ---------------- MLP phase (hop 0) ----------------
            for gi, sbs in enumerate(st.groups):
                nsb = len(sbs)
                n0 = sbs[0] * 128
                nn = sum(st.sb_ndst[s] for s in sbs)
                ps1 = qp.tile([64, 512], F32, tag="mlp1", bufs=2)
                for ch in range(4):
                    xt = sp.tile([128, 512], F32, tag="xt")
                    nc.sync.dma_start(
                        out=xt[:, :nn],
                        in_=xT[ch * 128:(ch + 1) * 128, n0:n0 + nn])
                    nc.tensor.matmul(
                        ps1[:, :nn], w1_t[:, ch * 64:(ch + 1) * 64],
                        xt[:, :nn], start=(ch == 0), stop=(ch == 3))
                h0t = sp.tile([64, 512], F32, tag="h0t")
                nc.scalar.activation(
                    h0t[:, :nn], ps1[:, :nn],
                    mybir.ActivationFunctionType.Relu)
                h_f32 = sp.tile([128, 8 * 64], F32, tag="hf")
                h_b16 = sp.tile([128, 8 * 64], BF16, tag="hb")
                for sl in range(nsb):
                    nd = st.sb_ndst[sbs[sl]]
                    ps2 = qp.tile([128, 64], F32, tag="mlp2", bufs=2)
                    nc.tensor.matmul(
                        ps2[:nd, :], h0t[:, sl * 128:sl * 128 + nd],
                        w2_t[:], start=True, stop=True)
                    nc.scalar.activation(
                        h_f32[:nd, sl * 64:(sl + 1) * 64], ps2[:nd, :],
                        mybir.ActivationFunctionType.Relu)
                    nc.scalar.activation(
                        h_b16[:nd, sl * 64:(sl + 1) * 64], ps2[:nd, :],
                        mybir.ActivationFunctionType.Relu)
                gate_and_bounce(h_f32, h_b16, gi, 0)

            # ---------------- hops ----------------
            for hop in range(1, hops + 1):
                if ag_mode == 'all' or (ag_mode == 'once' and hop == 1):
                    tbl = tables[(hop - 1) % 2] if ag_mode == 'all' else tables[0]
                    nc.gpsimd.collective_compute(
                        "AllGather", mybir.AluOpType.bypass,
                        replica_groups=replica,
                        ins=[bounces[(hop - 1) % 2][:]],
                        outs=[tbl[:]],
                    )
                else:
                    tbl = tables[0] if ag_mode == 'once' else tables[(hop - 1) % 2]
                for gi, sbs in enumerate(st.groups):
                    nsb = len(sbs)
                    wins = st.g_wins[gi]
                    nwin = len(wins)
                    ntile = nwin + st.r_tiles
                    nidx = ntile * 128
                    gbufs = []
                    for b in range(st.n_banks):
                        gb = sp.tile([128, st.max_tiles, 128], BF16,
                                     tag=f"gb{b}", bufs=3)
                        ic0 = (gi * st.n_banks + b) * st.idx_cols
                        gin = nc.gpsimd.dma_gather(
                            gb[:, :ntile, :],
                            tbl[b * st.bank_sz:
                                min((b + 1) * st.bank_sz, st.n_nodes), :],
                            idx_sb[:, ic0:ic0 + ntile * 8],
                            nidx, nidx, 128,
                            single_packet=False, queue_num=b)
                        add_dep_helper(gin.ins, ll.ins, sync=True,
                                       reason="lib")
                        gbufs.append(gb)
                    s_sb = sp.tile([128, st.s_cols], BF16, tag="ssb")
                    nc.sync.dma_start(
                        out=s_sb[:, :st.g_mms[gi] * 32],
                        in_=s_d[gi, :, :st.g_mms[gi] * 32])
                    pss = [qp.tile([128, 64], F32, tag="spmm", bufs=4,
                                   name=f"spmm{gi}_{i}")
                           for i in range(nsb)]
                    mm = 0

                    def emit(b, t, sl, woff, start, stop):
                        nonlocal mm
                        kw = {}
                        if woff == 96:
                            kw["tile_position"] = (0, 96)
                        nc.tensor.matmul(
                            pss[sl][woff:woff + 32, 0:64],
                            s_sb[:, mm * 32:(mm + 1) * 32],
                            gbufs[b][:, t, 0:64],
                            start=start, stop=stop,
                            skip_group_check=True, **kw)
                        mm += 1

                    for b in range(st.n_banks):
                        for k, (sl, _sbv, woff) in enumerate(wins):
                            emit(b, k, sl, woff, b == 0, False)
                    for rt in range(st.r_tiles):
                        for b in range(st.n_banks):
                            last = (rt == st.r_tiles - 1
                                    and b == st.n_banks - 1)
                            for k, (sl, _sbv, woff) in enumerate(wins):
                                emit(b, nwin + rt, sl, woff, False, last)

                    h_f32 = sp.tile([128, 8 * 64], F32, tag="hf")
                    h_b16 = sp.tile([128, 8 * 64], BF16, tag="hb")
                    for sl in range(nsb):
                        nc.scalar.copy(
                            h_f32[:, sl * 64:(sl + 1) * 64], pss[sl][:, :])
                        nc.scalar.copy(
                            h_b16[:, sl * 64:(sl + 1) * 64], pss[sl][:, :])
                    gate_and_bounce(h_f32, h_b16, gi, hop)

            # ---------------- output ----------------
            for sb in range(st.nsb):
                nd = st.sb_ndst[sb]
                nc.sync.dma_start(
                    out=out_d[sb * 128:sb * 128 + nd, :],
                    in_=acc[0:nd, sb * 64:(sb + 1) * 64])
    nc.compile()
    return nc


# ---------------------------------------------------------------------------
# runner (PJRT via axon shard_map; executable cached)
# ---------------------------------------------------------------------------
class SpmdRunner:
    def __init__(self, nc, n_cores):
        import jax
        from jax.sharding import Mesh, PartitionSpec, NamedSharding
        from jax.experimental.shard_map import shard_map
        from concourse import bass2jax

        bass2jax.install_neuronx_cc_hook()
        self.jax = jax
        self.nc = nc
        self.n_cores = n_cores
        partition_name = (
            nc.partition_id_tensor.name if nc.partition_id_tensor else None)
        in_names, out_names, out_avals = [], [], []
        for alloc in nc.m.functions[0].allocations:
            if not isinstance(alloc, mybir.MemoryLocationSet):
                continue
            name = alloc.memorylocations[0].name
            if alloc.kind == "ExternalInput":
                if name != partition_name and name != (
                        nc.dbg_addr.name if nc.dbg_addr else None):
                    in_names.append(name)
            elif alloc.kind == "ExternalOutput":
                out_names.append(name)
                out_avals.append(jax.core.ShapedArray(
                    tuple(alloc.tensor_shape), mybir.dt.np(alloc.dtype)))
        self.in_names, self.out_names, self.out_avals = (
            in_names, out_names, out_avals)
        n_params = len(in_names)
        bind_in_names = list(in_names) + list(out_names)
        self._has_dbg = nc.dbg_addr is not None
        if self._has_dbg:
            bind_in_names.append(nc.dbg_addr.name)
        if partition_name is not None:
            bind_in_names.append(partition_name)

        def _body(*args):
            operands = list(args)
            if partition_name is not None:
                operands.append(bass2jax.partition_id_tensor())
            outs = bass2jax._bass_exec_p.bind(
                *operands, out_avals=tuple(out_avals),
                in_names=tuple(bind_in_names), out_names=tuple(out_names),
                lowering_input_output_aliases=(),
                sim_require_finite=False, sim_require_nnan=False, nc=nc)
            return tuple(outs)

        n_extra = len(out_names) + (1 if self._has_dbg else 0)
        devices = jax.devices()[:n_cores]
        mesh = Mesh(np.asarray(devices), ("core",))
        self.in_sharding = NamedSharding(mesh, PartitionSpec("core"))
        self.jitted = jax.jit(
            shard_map(_body, mesh=mesh,
                      in_specs=(PartitionSpec("core"),) * (n_params + n_extra),
                      out_specs=(PartitionSpec("core"),) * len(out_names),
                      check_rep=False),
            keep_unused=True)

    def put_inputs(self, in_maps):
        jax = self.jax
        args = []
        for name in self.in_names:
            cat = np.concatenate(
                [np.ascontiguousarray(m[name]) for m in in_maps], axis=0)
            args.append(jax.device_put(cat, self.in_sharding))
        for av in self.out_avals:
            z = np.zeros((self.n_cores * av.shape[0], *av.shape[1:]),
                         av.dtype)
            args.append(jax.device_put(z, self.in_sharding))
        if self._has_dbg:
            args.append(jax.device_put(
                np.zeros((self.n_cores, 2), np.uint32), self.in_sharding))
        for a in args:
            a.block_until_ready()
        return args

    def run(self, args):
        out = self.jitted(*args)
        self.jax.block_until_ready(out)
        return out

    def outputs_per_core(self, out):
        res = []
        for c in range(self.n_cores):
            d = {}
            for i, name in enumerate(self.out_names):
                full = np.asarray(out[i])
                d[name] = full.reshape(
                    self.n_cores, *self.out_avals[i].shape)[c]
            res.append(d)
        return res


# ---------------------------------------------------------------------------
# entry point
# ---------------------------------------------------------------------------
_CACHE = {}


def _get_runner(st: Struct):
    key = (st.n_nodes, st.n_cores)
    if key not in _CACHE:
        nc = build_nc(st, st.hops)
        _CACHE[key] = SpmdRunner(nc, st.n_cores)
    return _CACHE[key]


def make_in_maps(st, x, edge_src, edge_dst, edge_weight, W1, W2, w_prop):
    in_maps = []
    wprop_b = np.tile(np.asarray(w_prop, np.float32).reshape(1, D_HID),
                      (128, 1))
    for c in range(st.n_cores):
        lo, hi = c * st.shard, (c + 1) * st.shard
        idx_all, s_all = prep_core(st, c, edge_src, edge_dst, edge_weight)
        in_maps.append({
            "xT": np.ascontiguousarray(x[lo:hi].T.astype(np.float32)),
            "W1": np.asarray(W1, np.float32),
            "W2": np.asarray(W2, np.float32),
            "wprop": wprop_b,
            "idx": idx_all,
            "S": s_all,
        })
    return in_maps


def kernel(x, edge_src, edge_dst, edge_weight, node_index, W1, W2, w_prop):
    x = np.asarray(x)
    edge_src = np.asarray(edge_src)
    edge_dst = np.asarray(edge_dst)
    edge_weight = np.asarray(edge_weight)
    node_index = np.asarray(node_index)
    st = Struct(x.shape[0], N_CORES)
    runner = _get_runner(st)
    in_maps = make_in_maps(st, x, edge_src, edge_dst, edge_weight,
                           W1, W2, w_prop)
    args = runner.put_inputs(in_maps)
    out = runner.run(args)
    per_core = runner.outputs_per_core(out)
    full = np.concatenate([pc["out"] for pc in per_core], axis=0)
    return full[node_index].astype(np.float32)



# revision 3
# speedup vs baseline: 1.0280x; 1.0280x over previous
"""DAGNN (MLP + 10-hop SpMM propagation + sigmoid-gated combine) on 8 trn2
NeuronCores via Bass/Tile.

Strategy:
  - dst-sharding: core c owns 12500 destination nodes.
  - MLP h0 = relu(relu(x@W1)@W2) computed per-core on its dst slice (PE, f32),
    x passed pre-transposed (host) so no on-device transposes are needed.
  - Node-feature table [100000, 128] bf16 (64 real features + 64 pad to reach
    the 256B dma_gather element size) replicated per-core in DRAM, rebuilt
    each hop with an AllGather of the 8 per-core slices.
  - Per hop: per-edge rows fetched with dma_gather (InstDMAGatherAnt, 4 SWDGE
    queues, one per 25000-row src bank so indices fit int16), then a one-hot
    weighted segment-sum on the TensorEngine: out[win] += S.T @ G with S
    [128 edges, 32 dsts] bf16 streamed from DRAM, accumulating f32 in PSUM
    windows at partition offsets {0,32,64,96}.
  - Canonical per-group structure (4 superblocks of 128 dsts): per bank one
    M-tile per 32-window plus two R-tiles for bucket overflow, so the
    instruction stream is identical across cores (SPMD) and only idx/S data
    differ.
  - Gating acc += sigmoid(h_k . w_prop) * h_k folded into each hop's PSUM
    eviction (DVE/ACT), acc kept f32 in SBUF.
  - Host gathers out[node_index] from the 8 returned slices.
"""

import math

import numpy as np
import ml_dtypes

import concourse.bass as bass
import concourse.bacc as bacc
import concourse.tile as tile
import concourse.mybir as mybir
from concourse import library_config
from concourse.tile import add_dep_helper

F32 = mybir.dt.float32
BF16 = mybir.dt.bfloat16
I16 = mybir.dt.int16

# problem constants (hardcoded per harness contract)
N_NODES = 100000
N_EDGES = 1600000
K_HOPS = 10
D_IN = 512
D_HID = 64
N_IDX = 10000
N_CORES = 8


# ---------------------------------------------------------------------------
# structure (shared across cores; sizes must be core-independent)
# ---------------------------------------------------------------------------
class Struct:
    def __init__(self, n_nodes, n_cores, n_banks=4, sb_per_group=4, r_tiles=2, hops=K_HOPS):
        self.n_nodes = n_nodes
        self.hops = hops
        self.n_cores = n_cores
        self.shard = n_nodes // n_cores
        assert self.shard * n_cores == n_nodes
        self.n_banks = n_banks
        self.nsb = int(math.ceil(self.shard / 128))
        self.bank_sz = int(math.ceil(n_nodes / n_banks))
        assert self.bank_sz <= 32768
        self.r_tiles = r_tiles
        # groups of superblocks
        self.groups = [
            list(range(g, min(g + sb_per_group, self.nsb)))
            for g in range(0, self.nsb, sb_per_group)
        ]
        # windows per sb: list of (sb, woff, ndst_in_window)
        self.sb_ndst = [
            min(128, self.shard - sb * 128) for sb in range(self.nsb)
        ]
        self.sb_wins = []
        for sb in range(self.nsb):
            wins = []
            nd = self.sb_ndst[sb]
            for woff in range(0, nd, 32):
                wins.append((woff, min(32, nd - woff)))
            self.sb_wins.append(wins)
        # per group: windows list [(sb_local, sb, woff)], tiles per bank
        self.g_wins = []
        self.g_tiles = []  # tiles per bank per group (nwin + r_tiles)
        for sbs in self.groups:
            wins = []
            for sl, sb in enumerate(sbs):
                for (woff, _n) in self.sb_wins[sb]:
                    wins.append((sl, sb, woff))
            self.g_wins.append(wins)
            self.g_tiles.append(len(wins) + r_tiles)
        self.max_tiles = max(self.g_tiles)
        # matmuls per group: banks * nwin (M) + banks * r_tiles * nwin (R)
        self.g_mms = [
            self.n_banks * len(w) * (1 + r_tiles) for w in self.g_wins
        ]
        self.max_mms = max(self.g_mms)
        self.s_cols = self.max_mms * 32  # S columns per group (padded)
        self.n_groups = len(self.groups)
        # idx tensor: per (group, bank) call: max_tiles*128 idxs -> /16 cols
        self.idx_cols = self.max_tiles * 8  # (tiles*128)/16


# ---------------------------------------------------------------------------
# host-side data prep
# ---------------------------------------------------------------------------
def prep_core(st: Struct, c, edge_src, edge_dst, edge_weight):
    """Build idx layout + S tiles for one core. Returns (idx_all, s_all)."""
    lo, hi = c * st.shard, (c + 1) * st.shard
    m = (edge_dst >= lo) & (edge_dst < hi)
    src = edge_src[m].astype(np.int64)
    dstl = (edge_dst[m] - lo).astype(np.int64)
    w = edge_weight[m].astype(np.float32)
    bank = src // st.bank_sz
    srcl = src - bank * st.bank_sz

    sb = dstl >> 7
    woff = (dstl & 127) & ~31  # 32-aligned offset within sb

    idx_all = np.zeros((st.n_groups, st.n_banks, 128, st.idx_cols), np.int16)
    s_all = np.zeros((st.n_groups, 128, st.s_cols), np.float32)

    # bucket edges by (group, bank, window-within-group)
    g_of_sb = np.zeros(st.nsb, np.int64)
    for gi, sbs in enumerate(st.groups):
        for s in sbs:
            g_of_sb[s] = gi
    g = g_of_sb[sb]

    order = np.lexsort((dstl, bank, g))
    src_o, dstl_o, w_o, bank_o, g_o = (
        srcl[order], dstl[order], w[order], bank[order], g[order])
    sb_o, woff_o = sb[order], woff[order]

    # boundaries per (g, bank)
    key = g_o * st.n_banks + bank_o
    bounds = np.searchsorted(key, np.arange(st.n_groups * st.n_banks + 1))

    for gi in range(st.n_groups):
        wins = st.g_wins[gi]
        win_pos = {(sbv, wo): k for k, (_sl, sbv, wo) in enumerate(wins)}
        nwin = len(wins)
        n_mm_g = st.n_banks * nwin * (1 + st.r_tiles)
        mm = 0
        # slot layout per bank: [win0 tile | win1 tile | ... | R0 | R1]
        for b in range(st.n_banks):
            a0, a1 = bounds[gi * st.n_banks + b], bounds[gi * st.n_banks + b + 1]
            es, ed, ew2 = src_o[a0:a1], dstl_o[a0:a1], w_o[a0:a1]
            esb, ewo = sb_o[a0:a1], woff_o[a0:a1]
            # per window lists
            rest_idx = []
            rest_dst = []
            rest_w = []
            tiles_idx = np.zeros((st.max_tiles, 128), np.int16)
            tiles_S = [None] * st.max_tiles  # (rows, cols, vals) per tile
            for k, (_sl, sbv, wo) in enumerate(wins):
                emask = (esb == sbv) & (ewo == wo)
                ii = np.nonzero(emask)[0]
                take = ii[:128]
                over = ii[128:]
                n = len(take)
                tiles_idx[k, :n] = es[take]
                tiles_S[k] = (np.arange(n), ed[take] - (sbv * 128 + wo),
                              ew2[take])
                if len(over):
                    rest_idx.append(es[over])
                    rest_dst.append(ed[over])
                    rest_w.append(ew2[over])
            if rest_idx:
                rest_idx = np.concatenate(rest_idx)
                rest_dst = np.concatenate(rest_dst)
                rest_w = np.concatenate(rest_w)
            else:
                rest_idx = np.zeros(0, np.int64)
                rest_dst = np.zeros(0, np.int64)
                rest_w = np.zeros(0, np.float32)
            assert len(rest_idx) <= 128 * st.r_tiles, (
                f"R overflow core{c} g{gi} b{b}: {len(rest_idx)}")
            for rt in range(st.r_tiles):
                rr = slice(rt * 128, min((rt + 1) * 128, len(rest_idx)))
                ridx = rest_idx[rr]
                k = nwin + rt
                tiles_idx[k, :len(ridx)] = ridx
                tiles_S[k] = (rest_dst[rr], rest_w[rr])

            # idx layout: tile t slot p -> linear j = t*128+p ->
            # [partition j%16 (replicated x8 later), col j//16]
            lin = tiles_idx.reshape(-1)  # [max_tiles*128]
            wrapped = lin.reshape(-1, 16).T  # [16, idx_cols]
            idx_all[gi, b] = np.tile(wrapped, (8, 1))

            # S blocks: M then (R x r_tiles), emission order bank-major
            # M blocks for this bank at mm positions: b*nwin + k
            for k in range(nwin):
                rows, cols, vals = tiles_S[k]
                pos = b * nwin + k
                blk = np.zeros((128, 32), np.float32)
                blk[rows, cols] = vals
                s_all[gi, :, pos * 32:(pos + 1) * 32] = blk
            # R blocks: position banks*nwin + (rt*banks + b)*nwin + k
            for rt in range(st.r_tiles):
                rdst, rw = tiles_S[nwin + rt]
                for k, (_sl, sbv, wo) in enumerate(wins):
                    pos = (st.n_banks * nwin
                           + (rt * st.n_banks + b) * nwin + k)
                    blk = np.zeros((128, 32), np.float32)
                    base = sbv * 128 + wo
                    sel = (rdst >= base) & (rdst < base + 32)
                    rr = np.nonzero(sel)[0]
                    blk[rr, rdst[rr] - base] = rw[rr]
                    s_all[gi, :, pos * 32:(pos + 1) * 32] = blk
        del n_mm_g, mm
    # idx layout for the single [128, G*B*C] SBUF load
    idx_flat = idx_all.transpose(2, 0, 1, 3).reshape(128, -1)
    return np.ascontiguousarray(idx_flat), s_all.astype(ml_dtypes.bfloat16)


# ---------------------------------------------------------------------------
# device program
# ---------------------------------------------------------------------------
def build_nc(st: Struct, hops: int = K_HOPS, ag_mode: str = 'all',
             shared_tables: bool = False):
    nc = bacc.Bacc(
        "TRN2", target_bir_lowering=False, debug=False, enable_asserts=False,
        num_devices=st.n_cores, num_swdge_queues=st.n_banks)

    shard = st.shard
    xT = nc.dram_tensor("xT", [D_IN, shard], F32, kind="ExternalInput")
    W1 = nc.dram_tensor("W1", [D_IN, D_HID], F32, kind="ExternalInput")
    W2 = nc.dram_tensor("W2", [D_HID, D_HID], F32, kind="ExternalInput")
    wprop = nc.dram_tensor("wprop", [128, D_HID], F32, kind="ExternalInput")
    idx_d = nc.dram_tensor(
        "idx", [128, st.n_groups * st.n_banks * st.idx_cols], I16,
        kind="ExternalInput")
    s_d = nc.dram_tensor(
        "S", [st.n_groups, 128, st.s_cols], BF16, kind="ExternalInput")
    out_d = nc.dram_tensor("out", [shard, D_HID], F32, kind="ExternalOutput")

    bounces = [nc.dram_tensor(f"bounce{p}", [shard, 128], BF16)
               for p in range(2)]
    tbl_kw = {"addr_space": "Shared"} if shared_tables else {}
    tables = [nc.dram_tensor(f"table{p}", [st.n_nodes, 128], BF16, **tbl_kw)
              for p in range(2)]

    replica = [list(range(st.n_cores))]

    with tile.TileContext(nc) as tc:
        with (
            tc.tile_pool(name="sbuf", bufs=2) as sp,
            tc.tile_pool(name="persist", bufs=1) as pp,
            tc.tile_pool(name="psum", bufs=2, space="PSUM") as qp,
        ):
            ll = nc.gpsimd.load_library(library_config.mlp)

            # persistent tiles
            acc = pp.tile([128, st.nsb * 64], F32, tag="acc")
            idx_sb = pp.tile([128, st.n_groups * st.n_banks * st.idx_cols],
                             I16, tag="idxs")
            wp_t = pp.tile([128, 64], F32, tag="wp")
            w1_t = pp.tile([128, 4 * 64], F32, tag="w1")
            w2_t = pp.tile([64, 64], F32, tag="w2")
            nc.sync.dma_start(out=wp_t[:], in_=wprop[:])
            nc.sync.dma_start(
                out=w1_t[:].rearrange("p (c d) -> p c d", d=64),
                in_=W1[:].rearrange("(c p) d -> p c d", p=128))
            nc.sync.dma_start(out=w2_t[:], in_=W2[:])
            nc.sync.dma_start(out=idx_sb[:], in_=idx_d[:])
            nc.vector.memset(acc[:], 0.0)

            def gate_and_bounce(h_f32, h_b16, gi, hop):
                """gating acc += sig(h.wp)*h ; write bounce slices (bf16)."""
                sbs = st.groups[gi]
                nsb = len(sbs)
                dot = sp.tile([128, 8], F32, tag="dot")
                sg = sp.tile([128, 8], F32, tag="sg")
                tmp = sp.tile([128, 8 * 64], F32, tag="gtmp")
                h3 = h_f32[:, 0:nsb * 64].rearrange("p (s f) -> p s f", f=64)
                nc.vector.tensor_tensor(
                    out=tmp[:, 0:nsb * 64].rearrange("p (s f) -> p s f", f=64),
                    in0=h3,
                    in1=wp_t[:].rearrange("p (o f) -> p o f",
                                          o=1).to_broadcast([128, nsb, 64]),
                    op=mybir.AluOpType.mult)
                nc.vector.tensor_reduce(
                    out=dot[:, :nsb],
                    in_=tmp[:, 0:nsb * 64].rearrange("p (s f) -> p s f",
                                                     f=64),
                    axis=mybir.AxisListType.X,
                    op=mybir.AluOpType.add)
                nc.scalar.activation(
                    sg[:, :nsb], dot[:, :nsb],
                    mybir.ActivationFunctionType.Sigmoid)
                nc.vector.tensor_tensor(
                    out=tmp[:, 0:nsb * 64].rearrange("p (s f) -> p s f", f=64),
                    in0=h3,
                    in1=sg[:, :nsb].rearrange("p (s o) -> p s o",
                                              o=1).to_broadcast(
                        [128, nsb, 64]),
                    op=mybir.AluOpType.mult)
                g0 = sbs[0]
                nc.vector.tensor_tensor(
                    out=acc[:, g0 * 64:(g0 + nsb) * 64],
                    in0=acc[:, g0 * 64:(g0 + nsb) * 64],
                    in1=tmp[:, 0:nsb * 64],
                    op=mybir.AluOpType.add)
                if hop < st.hops:
                    bnc = bounces[hop % 2]
                    for sl, sbv in enumerate(sbs):
                        nd = st.sb_ndst[sbv]
                        nc.sync.dma_start(
                            out=bnc[sbv * 128:sbv * 128 + nd, 0:64],
                            in_=h_b16[0:nd, sl * 64:(sl + 1) * 64])

            # ---------------- MLP phase (hop 0) ----------------
            for gi, sbs in enumerate(st.groups):
                nsb = len(sbs)
                n0 = sbs[0] * 128
                nn = sum(st.sb_ndst[s] for s in sbs)
                ps1 = qp.tile([64, 512], F32, tag="mlp1", bufs=2)
                for ch in range(4):
                    xt = sp.tile([128, 512], F32, tag="xt")
                    nc.sync.dma_start(
                        out=xt[:, :nn],
                        in_=xT[ch * 128:(ch + 1) * 128, n0:n0 + nn])
                    nc.tensor.matmul(
                        ps1[:, :nn], w1_t[:, ch * 64:(ch + 1) * 64],
                        xt[:, :nn], start=(ch == 0), stop=(ch == 3))
                h0t = sp.tile([64, 512], F32, tag="h0t")
                nc.scalar.activation(
                    h0t[:, :nn], ps1[:, :nn],
                    mybir.ActivationFunctionType.Relu)
                h_f32 = sp.tile([128, 8 * 64], F32, tag="hf")
                h_b16 = sp.tile([128, 8 * 64], BF16, tag="hb")
                for sl in range(nsb):
                    nd = st.sb_ndst[sbs[sl]]
                    ps2 = qp.tile([128, 64], F32, tag="mlp2", bufs=2)
                    nc.tensor.matmul(
                        ps2[:nd, :], h0t[:, sl * 128:sl * 128 + nd],
                        w2_t[:], start=True, stop=True)
                    nc.scalar.activation(
                        h_f32[:nd, sl * 64:(sl + 1) * 64], ps2[:nd, :],
                        mybir.ActivationFunctionType.Relu)
                    nc.scalar.activation(
                        h_b16[:nd, sl * 64:(sl + 1) * 64], ps2[:nd, :],
                        mybir.ActivationFunctionType.Relu)
                gate_and_bounce(h_f32, h_b16, gi, 0)

            # ---------------- hops ----------------
            for hop in range(1, hops + 1):
                if ag_mode == 'all' or (ag_mode == 'once' and hop == 1):
                    tbl = tables[(hop - 1) % 2] if ag_mode == 'all' else tables[0]
                    nc.gpsimd.collective_compute(
                        "AllGather", mybir.AluOpType.bypass,
                        replica_groups=replica,
                        ins=[bounces[(hop - 1) % 2][:]],
                        outs=[tbl[:]],
                    )
                else:
                    tbl = tables[0] if ag_mode == 'once' else tables[(hop - 1) % 2]
                for gi, sbs in enumerate(st.groups):
                    nsb = len(sbs)
                    wins = st.g_wins[gi]
                    nwin = len(wins)
                    ntile = nwin + st.r_tiles
                    nidx = ntile * 128
                    gbufs = []
                    for b in range(st.n_banks):
                        gb = sp.tile([128, st.max_tiles, 128], BF16,
                                     tag=f"gb{b}", bufs=3)
                        ic0 = (gi * st.n_banks + b) * st.idx_cols
                        gin = nc.gpsimd.dma_gather(
                            gb[:, :ntile, :],
                            tbl[b * st.bank_sz:
                                min((b + 1) * st.bank_sz, st.n_nodes), :],
                            idx_sb[:, ic0:ic0 + ntile * 8],
                            nidx, nidx, 128,
                            single_packet=False, queue_num=b)
                        add_dep_helper(gin.ins, ll.ins, sync=True,
                                       reason="lib")
                        gbufs.append(gb)
                    s_sb = sp.tile([128, st.s_cols], BF16, tag="ssb")
                    nc.sync.dma_start(
                        out=s_sb[:, :st.g_mms[gi] * 32],
                        in_=s_d[gi, :, :st.g_mms[gi] * 32])
                    pss = [qp.tile([128, 64], F32, tag="spmm", bufs=4,
                                   name=f"spmm{gi}_{i}")
                           for i in range(nsb)]
                    mm = 0

                    def emit(b, t, sl, woff, start, stop):
                        nonlocal mm
                        kw = {}
                        if woff == 96:
                            kw["tile_position"] = (0, 96)
                        nc.tensor.matmul(
                            pss[sl][woff:woff + 32, 0:64],
                            s_sb[:, mm * 32:(mm + 1) * 32],
                            gbufs[b][:, t, 0:64],
                            start=start, stop=stop,
                            skip_group_check=True, **kw)
                        mm += 1

                    for b in range(st.n_banks):
                        for k, (sl, _sbv, woff) in enumerate(wins):
                            emit(b, k, sl, woff, b == 0, False)
                    for rt in range(st.r_tiles):
                        for b in range(st.n_banks):
                            last = (rt == st.r_tiles - 1
                                    and b == st.n_banks - 1)
                            for k, (sl, _sbv, woff) in enumerate(wins):
                                emit(b, nwin + rt, sl, woff, False, last)

                    h_f32 = sp.tile([128, 8 * 64], F32, tag="hf")
                    h_b16 = sp.tile([128, 8 * 64], BF16, tag="hb")
                    for sl in range(nsb):
                        nc.scalar.copy(
                            h_f32[:, sl * 64:(sl + 1) * 64], pss[sl][:, :])
                        nc.scalar.copy(
                            h_b16[:, sl * 64:(sl + 1) * 64], pss[sl][:, :])
                    gate_and_bounce(h_f32, h_b16, gi, hop)

            # ---------------- output ----------------
            for sb in range(st.nsb):
                nd = st.sb_ndst[sb]
                nc.sync.dma_start(
                    out=out_d[sb * 128:sb * 128 + nd, :],
                    in_=acc[0:nd, sb * 64:(sb + 1) * 64])
    nc.compile()
    return nc


# ---------------------------------------------------------------------------
# runner (PJRT via axon shard_map; executable cached)
# ---------------------------------------------------------------------------
class SpmdRunner:
    def __init__(self, nc, n_cores):
        import jax
        from jax.sharding import Mesh, PartitionSpec, NamedSharding
        from jax.experimental.shard_map import shard_map
        from concourse import bass2jax

        bass2jax.install_neuronx_cc_hook()
        self.jax = jax
        self.nc = nc
        self.n_cores = n_cores
        partition_name = (
            nc.partition_id_tensor.name if nc.partition_id_tensor else None)
        in_names, out_names, out_avals = [], [], []
        for alloc in nc.m.functions[0].allocations:
            if not isinstance(alloc, mybir.MemoryLocationSet):
                continue
            name = alloc.memorylocations[0].name
            if alloc.kind == "ExternalInput":
                if name != partition_name and name != (
                        nc.dbg_addr.name if nc.dbg_addr else None):
                    in_names.append(name)
            elif alloc.kind == "ExternalOutput":
                out_names.append(name)
                out_avals.append(jax.core.ShapedArray(
                    tuple(alloc.tensor_shape), mybir.dt.np(alloc.dtype)))
        self.in_names, self.out_names, self.out_avals = (
            in_names, out_names, out_avals)
        n_params = len(in_names)
        bind_in_names = list(in_names) + list(out_names)
        self._has_dbg = nc.dbg_addr is not None
        if self._has_dbg:
            bind_in_names.append(nc.dbg_addr.name)
        if partition_name is not None:
            bind_in_names.append(partition_name)

        def _body(*args):
            operands = list(args)
            if partition_name is not None:
                operands.append(bass2jax.partition_id_tensor())
            outs = bass2jax._bass_exec_p.bind(
                *operands, out_avals=tuple(out_avals),
                in_names=tuple(bind_in_names), out_names=tuple(out_names),
                lowering_input_output_aliases=(),
                sim_require_finite=False, sim_require_nnan=False, nc=nc)
            return tuple(outs)

        n_extra = len(out_names) + (1 if self._has_dbg else 0)
        devices = jax.devices()[:n_cores]
        mesh = Mesh(np.asarray(devices), ("core",))
        self.in_sharding = NamedSharding(mesh, PartitionSpec("core"))
        self.jitted = jax.jit(
            shard_map(_body, mesh=mesh,
                      in_specs=(PartitionSpec("core"),) * (n_params + n_extra),
                      out_specs=(PartitionSpec("core"),) * len(out_names),
                      check_rep=False),
            keep_unused=True)

    def put_inputs(self, in_maps):
        jax = self.jax
        args = []
        for name in self.in_names:
            cat = np.concatenate(
                [np.ascontiguousarray(m[name]) for m in in_maps], axis=0)
            args.append(jax.device_put(cat, self.in_sharding))
        for av in self.out_avals:
            z = np.zeros((self.n_cores * av.shape[0], *av.shape[1:]),
                         av.dtype)
            args.append(jax.device_put(z, self.in_sharding))
        if self._has_dbg:
            args.append(jax.device_put(
                np.zeros((self.n_cores, 2), np.uint32), self.in_sharding))
        for a in args:
            a.block_until_ready()
        return args

    def run(self, args):
        out = self.jitted(*args)
        self.jax.block_until_ready(out)
        return out

    def outputs_per_core(self, out):
        res = []
        for c in range(self.n_cores):
            d = {}
            for i, name in enumerate(self.out_names):
                full = np.asarray(out[i])
                d[name] = full.reshape(
                    self.n_cores, *self.out_avals[i].shape)[c]
            res.append(d)
        return res


# ---------------------------------------------------------------------------
# entry point
# ---------------------------------------------------------------------------
_CACHE = {}


def _get_runner(st: Struct):
    key = (st.n_nodes, st.n_cores)
    if key not in _CACHE:
        nc = build_nc(st, st.hops)
        _CACHE[key] = SpmdRunner(nc, st.n_cores)
    return _CACHE[key]


def make_in_maps(st, x, edge_src, edge_dst, edge_weight, W1, W2, w_prop):
    in_maps = []
    wprop_b = np.tile(np.asarray(w_prop, np.float32).reshape(1, D_HID),
                      (128, 1))
    for c in range(st.n_cores):
        lo, hi = c * st.shard, (c + 1) * st.shard
        idx_all, s_all = prep_core(st, c, edge_src, edge_dst, edge_weight)
        in_maps.append({
            "xT": np.ascontiguousarray(x[lo:hi].T.astype(np.float32)),
            "W1": np.asarray(W1, np.float32),
            "W2": np.asarray(W2, np.float32),
            "wprop": wprop_b,
            "idx": idx_all,
            "S": s_all,
        })
    return in_maps


def kernel(x, edge_src, edge_dst, edge_weight, node_index, W1, W2, w_prop):
    x = np.asarray(x)
    edge_src = np.asarray(edge_src)
    edge_dst = np.asarray(edge_dst)
    edge_weight = np.asarray(edge_weight)
    node_index = np.asarray(node_index)
    st = Struct(x.shape[0], N_CORES)
    runner = _get_runner(st)
    in_maps = make_in_maps(st, x, edge_src, edge_dst, edge_weight,
                           W1, W2, w_prop)
    args = runner.put_inputs(in_maps)
    out = runner.run(args)
    per_core = runner.outputs_per_core(out)
    full = np.concatenate([pc["out"] for pc in per_core], axis=0)
    return full[node_index].astype(np.float32)

